# revision 1
# baseline (speedup 1.0000x reference)
"""Nearest-neighbor tokenizer on 8 Trainium2 NeuronCores.

Math: d2[t,m] = ||x_t||^2 + ||c_m||^2 - 2 x_t.c_m over 65536 tokens x 4096 codes.
out[t] = argmin_m d2 if min d2 <= 0.1 else -1.

Reformulated as g[t,m] = x_t.c_m - ||c_m||^2/2 (one K=65 GEMM with an
appended ones-row on x and a -c2/2 row on codes^T); then
min d2 = ||x_t||^2 - 2 max_m g, argmin d2 = argmax_m g.

Sharding: data-parallel over tokens. Core c gets batches [2c, 2c+2) ->
a contiguous slab of 8192 tokens; the codebook is replicated.
"""

import os

import numpy as np

B, N, D = 16, 4096, 64
M = 4096
NCORES = 8
TOK = B * N // NCORES          # 8192 tokens per core
NBLK = TOK // 128              # 64 blocks of 128 tokens
NCH = M // 512                 # 8 chunks of 512 codes
CBLK = M // 128                # 32 code blocks
THRESH = 0.1
FALLBACK_MARGIN = 2.0

_CACHE = {}


def _build(stage=6):
    import concourse.bacc as bacc
    import concourse.mybir as mybir
    import concourse.tile as tile
    from contextlib import ExitStack

    fp32 = mybir.dt.float32
    bf16 = mybir.dt.bfloat16
    i32 = mybir.dt.int32
    u32 = mybir.dt.uint32
    Alu = mybir.AluOpType
    Act = mybir.ActivationFunctionType

    nc = bacc.Bacc(
        "TRN2",
        target_bir_lowering=False,
        debug=False,
        enable_asserts=False,
        num_devices=1,
    )

    x_d = nc.dram_tensor("x", (TOK, D), fp32, kind="ExternalInput")
    c_d = nc.dram_tensor("codes", (M, D), fp32, kind="ExternalInput")
    id_d = nc.dram_tensor("ident", (128, 128), fp32, kind="ExternalInput")
    o_d = nc.dram_tensor("out", (TOK,), u32, kind="ExternalOutput")

    with tile.TileContext(nc) as tc, ExitStack() as ctx:
        sb = ctx.enter_context(tc.tile_pool(name="sb", bufs=1))

        ident = sb.tile((128, 128), fp32, tag="ident")
        xsb = sb.tile((128, NBLK, D), fp32, tag="xsb")
        csb = sb.tile((128, CBLK, D), fp32, tag="csb")
        xT = sb.tile((65, NBLK * 128), bf16, tag="xT")
        cT = sb.tile((65, M), bf16, tag="cT")
        cTsq = sb.tile((64, M), bf16, tag="cTsq")
        ones64 = sb.tile((64, 1), bf16, tag="ones64")
        x2 = sb.tile((128, NBLK), fp32, tag="x2")
        sq_all = sb.tile((128, NBLK, D), fp32, tag="sq_all")
        out_sb = sb.tile((128, NBLK), u32, tag="out_sb")
        top8 = sb.tile((128, 8), bf16, tag="top8")
        idx8 = sb.tile((128, 8), u32, tag="idx8")
        gmaxf = sb.tile((128, 1), fp32, tag="gmaxf")
        mind2 = sb.tile((128, 1), fp32, tag="mind2")
        mask = sb.tile((128, 1), mybir.dt.uint8, tag="mask")

        dma = nc.default_dma_engine
        dma.dma_start(out=ident, in_=id_d[:, :])
        dma.dma_start(out=xsb, in_=x_d[:, :].rearrange("(b p) d -> p b d", p=128))
        dma.dma_start(out=csb, in_=c_d[:, :].rearrange("(b p) d -> p b d", p=128))

        nc.vector.memset(xT[64:65, :], 1.0)
        nc.vector.memset(ones64, 1.0)
        nc.vector.memset(out_sb, 0xFFFFFFFF)

        # --- setup: transpose codes and x into [d, token/code] bf16 layout ---
        if stage >= 2:
            with tc.tile_pool(name="tpsum", bufs=4, space="PSUM") as tp:
                for cb in range(CBLK):
                    pt = tp.tile((64, 128), fp32, tag="ct")
                    nc.tensor.transpose(pt, csb[:, cb, :], ident)
                    nc.scalar.copy(cT[0:64, cb * 128:(cb + 1) * 128], pt)
                for xb in range(NBLK):
                    pt = tp.tile((64, 128), fp32, tag="xt")
                    nc.tensor.transpose(pt, xsb[:, xb, :], ident)
                    nc.scalar.copy(xT[0:64, xb * 128:(xb + 1) * 128], pt)

            # cTsq = cT*cT, c2 row: ones.T @ cTsq -> -c2/2 into cT row 64
            nc.vector.tensor_tensor(cTsq, cT[0:64, :], cT[0:64, :], op=Alu.mult)
            with tc.tile_pool(name="c2psum", bufs=2, space="PSUM") as cp:
                for j in range(NCH):
                    pt = cp.tile((1, 512), fp32, tag="c2")
                    nc.tensor.matmul(pt, ones64, cTsq[:, j * 512:(j + 1) * 512],
                                     start=True, stop=True)
                    nc.scalar.activation(cT[64:65, j * 512:(j + 1) * 512], pt,
                                         Act.Copy, bias=0.0, scale=-0.5)

        # x2[t] = sum_d x^2 (fp32): ACT square whole slab, DVE reduce innermost
        if stage >= 3:
            nc.scalar.activation(sq_all, xsb, Act.Square, bias=0.0, scale=1.0)
            nc.vector.tensor_reduce(x2, sq_all, axis=mybir.AxisListType.X,
                                    op=Alu.add)
        else:
            nc.vector.memset(x2, 1.0)

        # --- main loop ---
        if stage >= 4:
            with tc.tile_pool(name="gpsum", bufs=1, space="PSUM") as gp, \
                 tc.tile_pool(name="gsb", bufs=2) as gsb_pool:
                gbanks = [gp.tile((128, 512), fp32, tag=f"g{j}", name=f"g{j}")
                          for j in range(NCH)]
                for blk in range(NBLK):
                    lhsT = xT[:, blk * 128:(blk + 1) * 128]
                    g_sb = gsb_pool.tile((128, M), bf16, tag="g_sb")
                    for j in range(NCH):
                        nc.tensor.matmul(gbanks[j], lhsT,
                                         cT[:, j * 512:(j + 1) * 512],
                                         start=True, stop=True)
                        nc.scalar.copy(g_sb[:, j * 512:(j + 1) * 512], gbanks[j])
                    if stage >= 5:
                        nc.vector.max(top8, g_sb)
                        nc.vector.max_index(idx8, top8, g_sb)
                        nc.vector.tensor_copy(gmaxf, top8[:, 0:1])
                    if stage >= 6:
                        nc.vector.tensor_scalar(
                            out=mind2, in0=x2[:, blk:blk + 1],
                            scalar1=gmaxf[:, 0:1], scalar2=gmaxf[:, 0:1],
                            op0=Alu.subtract, op1=Alu.subtract)
                        nc.vector.tensor_scalar(
                            out=mask, in0=mind2, scalar1=THRESH, scalar2=None,
                            op0=Alu.is_le)
                        nc.vector.copy_predicated(out_sb[:, blk:blk + 1], mask,
                                                  idx8[:, 0:1])

        dma.dma_start(out=o_d[:].rearrange("(b p) -> p b", p=128), in_=out_sb)

    nc.compile()
    return nc


def _build_fast():
    """mind2-only program: no argmax. Per block: 8 matmuls -> PSUM; ACT
    evacuates banks 0-3 to bf16 SBUF, DVE folds banks 4&5 and 6&7 directly
    from PSUM; DVE TT-max tournament + tensor_reduce -> gmax[:, blk].
    mind2 = x2 - 2*gmax batched at the end. Output: mind2 fp32 (TOK,)."""
    import concourse.bacc as bacc
    import concourse.mybir as mybir
    import concourse.tile as tile
    from contextlib import ExitStack

    fp32 = mybir.dt.float32
    bf16 = mybir.dt.bfloat16
    Alu = mybir.AluOpType
    Act = mybir.ActivationFunctionType

    nc = bacc.Bacc(
        "TRN2",
        target_bir_lowering=False,
        debug=False,
        enable_asserts=False,
        num_devices=1,
    )

    x_d = nc.dram_tensor("x", (TOK, D), fp32, kind="ExternalInput")
    c_d = nc.dram_tensor("codes", (M, D), fp32, kind="ExternalInput")
    id_d = nc.dram_tensor("ident", (128, 128), fp32, kind="ExternalInput")
    o_d = nc.dram_tensor("mind2", (TOK,), fp32, kind="ExternalOutput")

    with tile.TileContext(nc) as tc, ExitStack() as ctx:
        sb = ctx.enter_context(tc.tile_pool(name="sb", bufs=1))

        ident = sb.tile((128, 128), fp32, tag="ident")
        xsb = sb.tile((128, NBLK, D), fp32, tag="xsb")
        csb = sb.tile((128, CBLK, D), fp32, tag="csb")
        xT = sb.tile((65, NBLK * 128), bf16, tag="xT")
        cT = sb.tile((65, M), bf16, tag="cT")
        cTsq = sb.tile((64, M), bf16, tag="cTsq")
        ones64 = sb.tile((64, 1), bf16, tag="ones64")
        x2 = sb.tile((128, NBLK), fp32, tag="x2")
        sq_all = sb.tile((128, NBLK, D), fp32, tag="sq_all")
        gmax = sb.tile((128, NBLK), fp32, tag="gmax")
        m2sb = sb.tile((128, NBLK), fp32, tag="m2sb")

        dma = nc.default_dma_engine
        dma.dma_start(out=ident, in_=id_d[:, :])
        dma.dma_start(out=xsb, in_=x_d[:, :].rearrange("(b p) d -> p b d", p=128))
        dma.dma_start(out=csb, in_=c_d[:, :].rearrange("(b p) d -> p b d", p=128))

        nc.vector.memset(xT[64:65, :], 1.0)
        nc.vector.memset(ones64, 1.0)

        with tc.tile_pool(name="tpsum", bufs=4, space="PSUM") as tp:
            for cb in range(CBLK):
                pt = tp.tile((64, 128), fp32, tag="ct")
                nc.tensor.transpose(pt, csb[:, cb, :], ident)
                nc.scalar.copy(cT[0:64, cb * 128:(cb + 1) * 128], pt)
            for xb in range(NBLK):
                pt = tp.tile((64, 128), fp32, tag="xt")
                nc.tensor.transpose(pt, xsb[:, xb, :], ident)
                nc.vector.tensor_copy(xT[0:64, xb * 128:(xb + 1) * 128], pt)

        nc.vector.tensor_tensor(cTsq, cT[0:64, :], cT[0:64, :], op=Alu.mult)
        with tc.tile_pool(name="c2psum", bufs=2, space="PSUM") as cp:
            for j in range(NCH):
                pt = cp.tile((1, 512), fp32, tag="c2")
                nc.tensor.matmul(pt, ones64, cTsq[:, j * 512:(j + 1) * 512],
                                 start=True, stop=True)
                nc.scalar.activation(cT[64:65, j * 512:(j + 1) * 512], pt,
                                     Act.Copy, bias=0.0, scale=-0.5)

        nc.scalar.activation(sq_all, xsb, Act.Square, bias=0.0, scale=1.0)
        nc.vector.tensor_reduce(x2, sq_all, axis=mybir.AxisListType.X,
                                op=Alu.add)

        with tc.tile_pool(name="gpsum", bufs=1, space="PSUM") as gp, \
             tc.tile_pool(name="tsb", bufs=3) as tpool:
            gbanks = [gp.tile((128, 512), fp32, tag=f"g{j}", name=f"g{j}")
                      for j in range(NCH)]
            for blk in range(NBLK):
                lhsT = xT[:, blk * 128:(blk + 1) * 128]
                g6 = tpool.tile((128, 6, 512), bf16, tag="g6")
                t2 = tpool.tile((128, 2, 512), bf16, tag="t2")
                m2 = tpool.tile((128, 2, 512), bf16, tag="m2")
                q2 = tpool.tile((128, 2, 512), bf16, tag="q2")
                r1 = tpool.tile((128, 512), bf16, tag="r1")
                for j in range(NCH):
                    nc.tensor.matmul(gbanks[j], lhsT,
                                     cT[:, j * 512:(j + 1) * 512],
                                     start=True, stop=True)
                for j in range(6):
                    nc.scalar.copy(g6[:, j, :], gbanks[j])
                # DVE may read at most one PSUM operand per instruction:
                # fold banks 6/7 against already-evacuated SBUF strips.
                nc.vector.tensor_tensor(t2[:, 0, :], gbanks[6], g6[:, 4, :],
                                        op=Alu.max)
                nc.vector.tensor_tensor(t2[:, 1, :], gbanks[7], g6[:, 5, :],
                                        op=Alu.max)
                nc.vector.tensor_tensor(m2, g6[:, 0:2, :], g6[:, 2:4, :],
                                        op=Alu.max)
                nc.vector.tensor_tensor(q2, m2, t2, op=Alu.max)
                nc.vector.tensor_tensor(r1, q2[:, 0, :], q2[:, 1, :],
                                        op=Alu.max)
                nc.vector.tensor_reduce(gmax[:, blk:blk + 1], r1,
                                        axis=mybir.AxisListType.X, op=Alu.max)

        nc.vector.tensor_scalar(out=m2sb, in0=gmax, scalar1=-2.0, scalar2=None,
                                op0=Alu.mult)
        nc.vector.tensor_tensor(m2sb, m2sb, x2, op=Alu.add)
        dma.dma_start(out=o_d[:].rearrange("(b p) -> p b", p=128), in_=m2sb)

    nc.compile()
    return nc


def _run(nc, in_maps, trace):
    from concourse import bass_utils
    try:
        return bass_utils.run_bass_kernel_spmd(
            nc, in_maps, list(range(NCORES)), trace=trace)
    except Exception:
        if not trace:
            raise
        return bass_utils.run_bass_kernel_spmd(
            nc, in_maps, list(range(NCORES)), trace=False)


def kernel(x: np.ndarray, codes: np.ndarray) -> np.ndarray:
    os.environ.setdefault("NEURON_RT_RESET_CORES", "1")
    x = np.ascontiguousarray(x, dtype=np.float32)
    codes = np.ascontiguousarray(codes, dtype=np.float32)
    ident = np.eye(128, dtype=np.float32)
    xf = x.reshape(NCORES, TOK, D)
    in_maps = [
        {"x": xf[c], "codes": codes, "ident": ident}
        for c in range(NCORES)
    ]
    trace = bool(os.environ.get("KERNEL_TRACE"))

    if os.environ.get("KERNEL_FORCE_FULL"):
        if "full" not in _CACHE:
            _CACHE["full"] = _build(6)
        res = _run(_CACHE["full"], in_maps, trace)
        _CACHE["last_res"] = res
        out = np.concatenate(
            [np.asarray(res.results[c]["out"], dtype=np.uint32)
             for c in range(NCORES)])
        return out.reshape(B, N).view(np.int32)

    if "fast" not in _CACHE:
        _CACHE["fast"] = _build_fast()
    res = _run(_CACHE["fast"], in_maps, trace)
    _CACHE["last_res"] = res
    mind2 = np.concatenate(
        [np.asarray(res.results[c]["mind2"], dtype=np.float32)
         for c in range(NCORES)])
    if mind2.min() > FALLBACK_MARGIN:
        return np.full((B, N), -1, dtype=np.int32)

    if "full" not in _CACHE:
        _CACHE["full"] = _build(6)
    res2 = _run(_CACHE["full"], in_maps, trace)
    out = np.concatenate(
        [np.asarray(res2.results[c]["out"], dtype=np.uint32)
         for c in range(NCORES)])
    return out.reshape(B, N).view(np.int32)



# revision 2
# speedup vs baseline: 1.8590x; 1.8590x over previous
"""Nearest-neighbor tokenizer on 8 Trainium2 NeuronCores.

Math: d2[t,m] = ||x_t||^2 + ||c_m||^2 - 2 x_t.c_m over 65536 tokens x 4096
codes; out[t] = argmin_m d2 if min d2 <= 0.1 else -1.

Device strategy (group screen): host pairs the 4096 codes into 2048 groups
(centroid mu_G, radius r_G).  Triangle inequality: min_{m in G} d(x, c_m)
>= d(x, mu_G) - r_G, so token t is provably code-free when for every G
    d(x, mu_G) > r_G + sqrt(0.1)
<=> h[t,G] := x.mu_G + ((r_G+thr)^2 - ||mu_G||^2)/2 < ||x||^2/2.

Each core takes 8192 tokens and evaluates h via one bf16 GEMM
([65,128] x [65,2048] per 128-token block: 64 centroid rows + bias row).
PSUM evacuation is split between the only two engines with a PSUM port:
DVE drains groups [0,896) with a fused tensor_reduce max -> pmax, and ACT
drains groups [896,2048) with Relu(h - (x2/2 - delta)) + accumulate -> S.
Token certified iff pmax + delta <= x2/2 and S == 0 (delta covers bf16
error).  The handful of uncertified tokens (1 for the benchmark input) are
refined exactly on the host; that path alone is fully correct for ANY
input, the device screen only prunes it.

Sharding: data-parallel over tokens, 8192 tokens/core; groups replicated.
"""

import os

import numpy as np

B, N, D = 16, 4096, 64
M = 4096
G = 2048                        # code groups (pairs)
NCORES = 8
TOK = B * N // NCORES           # 8192 tokens per core
NBLK = TOK // 128               # 64 blocks of 128 tokens
NCH = G // 512                  # 4 PSUM banks of 512 groups
DVE_W = 896                     # groups drained by DVE (exact max)
ACT_W = G - DVE_W               # groups drained by ACT (relu accum)
NXCHUNK = 8                     # input DMA chunks for overlap
THRESH = 0.1
DELTA = 0.75                    # certificate slack for bf16 device error
REFINE_CAP = 4000               # above this, refine everything on host

_CACHE = {}


def _build():
    import concourse.bacc as bacc
    import concourse.mybir as mybir
    import concourse.tile as tile
    from contextlib import ExitStack

    fp32 = mybir.dt.float32
    bf16 = mybir.dt.bfloat16
    Alu = mybir.AluOpType
    Act = mybir.ActivationFunctionType

    nc = bacc.Bacc(
        "TRN2",
        target_bir_lowering=False,
        debug=False,
        enable_asserts=False,
        num_devices=1,
    )

    xT_d = nc.dram_tensor("xT", (65, TOK), bf16, kind="ExternalInput")
    cT_d = nc.dram_tensor("cT", (65, G), bf16, kind="ExternalInput")
    nx2_d = nc.dram_tensor("nx2", (128, NBLK), fp32, kind="ExternalInput")
    pmax_d = nc.dram_tensor("pmax", (128, NBLK), fp32, kind="ExternalOutput")
    sact_d = nc.dram_tensor("sact", (128, NBLK), fp32, kind="ExternalOutput")

    with tile.TileContext(nc) as tc, ExitStack() as ctx:
        sb = ctx.enter_context(tc.tile_pool(name="sb", bufs=1))

        xsb = sb.tile((65, NBLK, 128), bf16, tag="xsb")
        csb = sb.tile((65, G), bf16, tag="csb")
        nx2 = sb.tile((128, NBLK), fp32, tag="nx2")
        pmax = sb.tile((128, NBLK), fp32, tag="pmax")
        sact = sb.tile((128, NBLK), fp32, tag="sact")

        dma = nc.default_dma_engine
        dma.dma_start(out=csb, in_=cT_d[:, :])
        dma.dma_start(out=nx2, in_=nx2_d[:, :])
        bpc = NBLK // NXCHUNK
        for ch in range(NXCHUNK):
            dma.dma_start(
                out=xsb[:, ch * bpc:(ch + 1) * bpc, :],
                in_=xT_d[:, ch * bpc * 128:(ch + 1) * bpc * 128],
            )

        with tc.tile_pool(name="gp", bufs=2, space="PSUM") as gp, \
             tc.tile_pool(name="scr", bufs=2) as scr:
            for blk in range(NBLK):
                g = gp.tile((128, G), fp32, tag="g")
                lhsT = xsb[:, blk, :]
                for j in range(NCH):
                    nc.tensor.matmul(g[:, j * 512:(j + 1) * 512], lhsT,
                                     csb[:, j * 512:(j + 1) * 512],
                                     start=True, stop=True)
                nc.vector.tensor_reduce(pmax[:, blk:blk + 1], g[:, 0:DVE_W],
                                        axis=mybir.AxisListType.X, op=Alu.max)
                s_out = scr.tile((128, ACT_W), bf16, tag="s")
                nc.scalar.activation(s_out, g[:, DVE_W:G], Act.Relu,
                                     bias=nx2[:, blk:blk + 1], scale=1.0,
                                     accum_out=sact[:, blk:blk + 1])

        dma.dma_start(out=pmax_d[:, :], in_=pmax)
        dma.dma_start(out=sact_d[:, :], in_=sact)

    nc.compile()
    return nc


def _pair_codes(codes):
    """Greedy nearest-neighbor pairing of the M codes into M/2 pairs.
    Returns (mu [G,64] f64, r [G] f64, bias [G] f64)."""
    c = codes.astype(np.float64)
    c2 = (c * c).sum(1)
    D2 = c2[:, None] + c2[None, :] - 2.0 * (c @ c.T)
    np.fill_diagonal(D2, np.inf)
    unmatched = np.ones(len(c), bool)
    pa, pb = [], []
    for i in range(len(c)):
        if not unmatched[i]:
            continue
        unmatched[i] = False
        row = np.where(unmatched, D2[i], np.inf)
        j = int(np.argmin(row))
        unmatched[j] = False
        pa.append(i)
        pb.append(j)
    pa = np.array(pa)
    pb = np.array(pb)
    mu = (c[pa] + c[pb]) * 0.5
    r = np.maximum(np.linalg.norm(c[pa] - mu, axis=1),
                   np.linalg.norm(c[pb] - mu, axis=1))
    thr = np.sqrt(THRESH)
    bias = ((r + thr) ** 2 - (mu * mu).sum(1)) * 0.5
    return mu, r, bias


def _refine(x_flat, codes, idxs):
    """Exact reference math for the given token indices."""
    c = codes.astype(np.float64)
    c2 = (c * c).sum(1)
    xs = x_flat[idxs].astype(np.float64)
    d2 = (xs * xs).sum(1)[:, None] + c2[None, :] - 2.0 * (xs @ c.T)
    nn = np.argmin(d2, axis=1).astype(np.int32)
    within = d2.min(1) <= THRESH
    return np.where(within, nn, np.int32(-1))


def kernel(x: np.ndarray, codes: np.ndarray) -> np.ndarray:
    import ml_dtypes
    from concourse import bass_utils

    os.environ.setdefault("NEURON_RT_RESET_CORES", "1")
    bf16 = ml_dtypes.bfloat16

    x = np.ascontiguousarray(x, dtype=np.float32)
    codes = np.ascontiguousarray(codes, dtype=np.float32)
    x_flat = x.reshape(-1, D)

    mu, r, bias = _pair_codes(codes)

    cT = np.empty((65, G), dtype=bf16)
    cT[0:64] = mu.T.astype(bf16)
    cT[64] = bias.astype(bf16)

    x2 = (x_flat.astype(np.float64) ** 2).sum(1)          # [65536]
    in_maps = []
    for c in range(NCORES):
        sl = slice(c * TOK, (c + 1) * TOK)
        xT = np.empty((65, TOK), dtype=bf16)
        xT[0:64] = x_flat[sl].T.astype(bf16)
        xT[64] = bf16(1.0)
        # token t = blk*128 + p  ->  nx2[p, blk] = delta - x2[t]/2
        nx2 = (DELTA - 0.5 * x2[sl]).astype(np.float32)
        nx2 = nx2.reshape(NBLK, 128).T.copy()
        in_maps.append({"xT": xT, "cT": cT, "nx2": nx2})

    if "nc" not in _CACHE:
        _CACHE["nc"] = _build()
    trace = bool(os.environ.get("KERNEL_TRACE"))
    try:
        res = bass_utils.run_bass_kernel_spmd(
            _CACHE["nc"], in_maps, list(range(NCORES)), trace=trace)
    except Exception:
        if not trace:
            raise
        res = bass_utils.run_bass_kernel_spmd(
            _CACHE["nc"], in_maps, list(range(NCORES)), trace=False)
    _CACHE["last_res"] = res

    pmax = np.concatenate(
        [np.asarray(res.results[c]["pmax"], dtype=np.float32).T.reshape(-1)
         for c in range(NCORES)])                          # [65536] token order
    sact = np.concatenate(
        [np.asarray(res.results[c]["sact"], dtype=np.float32).T.reshape(-1)
         for c in range(NCORES)])

    # Certificate: all groups' h below x2/2 (DVE side checked on host with
    # DELTA slack, ACT side baked the slack into its bias).
    ok = (pmax + DELTA <= 0.5 * x2) & (sact == 0.0)
    out = np.full(B * N, -1, dtype=np.int32)
    bad = np.flatnonzero(~ok)
    if len(bad) > REFINE_CAP:
        bad = np.arange(B * N)
    if len(bad):
        out[bad] = _refine(x_flat, codes, bad)
    return out.reshape(B, N)
